# revision 57
# baseline (speedup 1.0000x reference)
"""Trainium2 Bass kernel for nn_Metric_42674795053594 (Relation Network loss).

Self-contained: hardcodes all shapes. Shards batch b=8 across 8 NeuronCores
(1 episode/core), replicates params, uses AllGather for training-mode
BatchNorm statistics that couple all episodes.

v2: bf16 matmuls throughout (conv3/4, g-MLP, f-MLP), 2-stage maxpool split
across DVE+Pool engines, per-pair stats overlapped with conv, BN applies on
DVE tensor_scalar (4x mode), engine-balanced g-MLP epilogues.
"""
import sys, os
sys.path.insert(0, '/opt/trn_rl_repo')
import numpy as np
import ml_dtypes

import concourse.bass as bass
import concourse.mybir as mybir
import concourse.tile as tile
from concourse import bacc
from concourse.bass_utils import run_bass_kernel_spmd

F32 = mybir.dt.float32
BF16 = mybir.dt.bfloat16
FP8 = mybir.dt.float8e4
# conv2 kernel-tap pairs for fp8 DoubleRow: (j_a, j_b) with uniform
# in-pair column stride; j -> shift (j//3)*41 + j%3 on the 41-wide grid
C2_GROUPS = [(0, 1), (3, 4), (6, 7), (2, 5), (8, 8)]
AF = mybir.ActivationFunctionType
ALU = mybir.AluOpType
AX = mybir.AxisListType

EPS = 1e-5
NCORES = 8
S, Q = 5, 30
NPAIR = 18          # 36 image slots (5 sup + 30 qry + 1 pad) packed 2/partition-half
IMGW = 84
PLANE = 7232        # padded per-channel plane stride (>= 7056 + 170)
W1 = 7056           # conv1 output width (84*84)
PW1, PW2 = 1681, 361   # pooled widths: 41*41, 19*19
PW3 = 289              # 17*17 repacked width for conv4
PAD1, PAD2 = 88, 40    # shift-overshoot pads


def _bn_scalar_ops(nc, pool, s_ap, q_ap, n_elems, g_ap, b_ap, sc_out, sh_out, eps_ap, tag):
    """Given sum (s_ap) and sumsq (q_ap) APs [P,1], counts, gamma/beta APs,
    write scale into sc_out and shift into sh_out ([P,1])."""
    P = s_ap.shape[0]
    t = pool.tile([128, 4], F32, tag=f"bns_{tag}")
    mean, ex2, var, m2 = t[:P, 0:1], t[:P, 1:2], t[:P, 2:3], t[:P, 3:4]
    nc.vector.tensor_scalar_mul(mean, s_ap, 1.0 / n_elems)
    nc.vector.tensor_scalar_mul(ex2, q_ap, 1.0 / n_elems)
    nc.vector.tensor_tensor(m2, mean, mean, ALU.mult)
    nc.vector.tensor_tensor(var, ex2, m2, ALU.subtract)
    # sd = sqrt(var + eps); inv = 1/sd
    nc.scalar.activation(var, var, AF.Sqrt, bias=eps_ap)
    nc.vector.reciprocal(var, var)
    nc.vector.tensor_tensor(sc_out, g_ap, var, ALU.mult)
    nc.vector.tensor_tensor(m2, mean, sc_out, ALU.mult)
    nc.vector.tensor_tensor(sh_out, b_ap, m2, ALU.subtract)


def build_nc(n_cores=NCORES, debug=False):
    nc = bacc.Bacc("TRN2", target_bir_lowering=False, debug=False, num_devices=n_cores)
    RG = [list(range(n_cores))]

    # ---------------- I/O ----------------
    imgsr_d = nc.dram_tensor("imgsr", [18, 54, PLANE], BF16, kind="ExternalInput")
    w1t_d = nc.dram_tensor("w1t", [54, 128], BF16, kind="ExternalInput")
    wdr_d = nc.dram_tensor("wdr", [128, 5, 2, 128], FP8, kind="ExternalInput")
    wdr34_d = nc.dram_tensor("wdr34", [128, 2, 5, 2, 128], FP8, kind="ExternalInput")
    bng_d = nc.dram_tensor("bng", [64, 4], F32, kind="ExternalInput")
    bnb_d = nc.dram_tensor("bnb", [64, 4], F32, kind="ExternalInput")
    gw1s_d = nc.dram_tensor("gw1s", [66, 256], BF16, kind="ExternalInput")
    gw1q_d = nc.dram_tensor("gw1q", [66, 256], BF16, kind="ExternalInput")
    gb1_d = nc.dram_tensor("gb1t", [128, 2], F32, kind="ExternalInput")
    gwt_d = nc.dram_tensor("gwt", [128, 3, 2, 256], BF16, kind="ExternalInput")
    gbt_d = nc.dram_tensor("gbt", [128, 3, 2], F32, kind="ExternalInput")
    fwt_d = nc.dram_tensor("fwt", [128, 2, 2, 256], BF16, kind="ExternalInput")
    fbt_d = nc.dram_tensor("fbt", [128, 2, 2], F32, kind="ExternalInput")
    fw3_d = nc.dram_tensor("fw3t", [128, 2, 64], BF16, kind="ExternalInput")
    fb3_d = nc.dram_tensor("fb3t", [64, 1], F32, kind="ExternalInput")
    fw4_d = nc.dram_tensor("fw4t", [64, 1], BF16, kind="ExternalInput")
    fb4_d = nc.dram_tensor("fb4t", [1, 1], F32, kind="ExternalInput")
    fbng_d = nc.dram_tensor("fbng", [128, 2], F32, kind="ExternalInput")
    fbnb_d = nc.dram_tensor("fbnb", [128, 2], F32, kind="ExternalInput")
    coord45_d = nc.dram_tensor("coord45", [2, 45], BF16, kind="ExternalInput")
    coord270_d = nc.dram_tensor("coord270", [2, 270], BF16, kind="ExternalInput")
    lbl_d = nc.dram_tensor("lbl", [1, 150], F32, kind="ExternalInput")
    apmask_d = nc.dram_tensor("apmask", [1, 150], F32, kind="ExternalInput")

    loss_d = nc.dram_tensor("loss_part", [1, 1], F32, kind="ExternalOutput")
    if debug:
        feats_dbg_d = nc.dram_tensor("feats_dbg", [66, 324], F32, kind="ExternalOutput")
        dist_dbg_d = nc.dram_tensor("dist_dbg", [1, 150], F32, kind="ExternalOutput")
        xf_dbg_d = nc.dram_tensor("xf_dbg", [128, 2, 150], F32, kind="ExternalOutput")

    with tile.TileContext(nc) as tc:
        with (
            tc.tile_pool(name="pers", bufs=1) as pers,
            tc.tile_pool(name="dram", bufs=1, space="DRAM") as dram,
        ):
            # ---------------- load persistent weights ----------------
            w1t = pers.tile([54, 128], BF16)
            nc.sync.dma_start(w1t[:], w1t_d[:])
            wdr = pers.tile([128, 5, 2, 128], FP8)
            wdr34 = pers.tile([128, 2, 5, 2, 128], FP8)
            bng128 = pers.tile([128, 4], F32)
            nc.sync.dma_start(bng128[0:64, :], bng_d[:])
            nc.sync.dma_start(bng128[64:128, :], bng_d[:])
            bnb128 = pers.tile([128, 4], F32)
            nc.sync.dma_start(bnb128[0:64, :], bnb_d[:])
            nc.sync.dma_start(bnb128[64:128, :], bnb_d[:])
            gw1s = pers.tile([66, 256], BF16)
            gw1q = pers.tile([66, 256], BF16)
            gb1 = pers.tile([128, 2], F32)
            nc.sync.dma_start(gb1[:], gb1_d[:])
            gwt = pers.tile([128, 3, 2, 256], BF16)
            gbt = pers.tile([128, 3, 2], F32)
            nc.sync.dma_start(gbt[:], gbt_d[:])
            fwt = pers.tile([128, 2, 2, 256], BF16)
            fbt = pers.tile([128, 2, 2], F32)
            nc.sync.dma_start(fbt[:], fbt_d[:])
            fw3 = pers.tile([128, 2, 64], BF16)
            nc.sync.dma_start(fw3[:], fw3_d[:])
            fb3 = pers.tile([64, 1], F32)
            nc.sync.dma_start(fb3[:], fb3_d[:])
            fw4 = pers.tile([64, 1], BF16)
            nc.sync.dma_start(fw4[:], fw4_d[:])
            fb4 = pers.tile([1, 1], F32)
            nc.sync.dma_start(fb4[:], fb4_d[:])
            fbng = pers.tile([128, 2], F32)
            nc.sync.dma_start(fbng[:], fbng_d[:])
            fbnb = pers.tile([128, 2], F32)
            nc.sync.dma_start(fbnb[:], fbnb_d[:])
            lbl_sb = pers.tile([1, 150], F32)
            nc.sync.dma_start(lbl_sb[:], lbl_d[:])
            apmask_sb = pers.tile([1, 150], F32)
            nc.sync.dma_start(apmask_sb[:], apmask_d[:])

            epsc = pers.tile([128, 1], F32)
            nc.gpsimd.memset(epsc[:], EPS)
            margin = pers.tile([1, 1], F32)
            nc.gpsimd.memset(margin[:], 0.2)

            # persistent activations
            pooled2 = pers.tile([128, NPAIR * PW2 + PAD2], BF16)
            nc.gpsimd.memset(pooled2[:, NPAIR * PW2:], 0.0)
            p2q = pers.tile([128, NPAIR * PW2 + PAD2], FP8)
            nc.gpsimd.memset(p2q[:, NPAIR * PW2:], 0.0)
            feats = pers.tile([66, 324], BF16)
            nc.sync.dma_start(feats[64:66, 0:45], coord45_d[:])
            nc.sync.dma_start(feats[64:66, 45:315], coord270_d[:])
            # BN scale/shift per conv layer: [128, 2]
            # col 0 = [sup(top);qry(bot)] (pairs 0-4), col 1 = [qry;qry] (pairs 5-17)
            sc_t = [pers.tile([128, 2], F32, tag=f"sc{l}", name=f"sc{l}") for l in range(4)]
            sh_t = [pers.tile([128, 2], F32, tag=f"sh{l}", name=f"sh{l}") for l in range(4)]
            xf = pers.tile([128, 2, 150], F32)

            # stat accumulators per conv layer
            sumacc = [pers.tile([128, NPAIR], F32, tag=f"sma{l}", name=f"sma{l}") for l in range(4)]
            sqacc = [pers.tile([128, NPAIR], F32, tag=f"sqa{l}", name=f"sqa{l}") for l in range(4)]
            # scratch sinks for stat passes (bf16, max width PW1)
            tssc = pers.tile([128, PW1], BF16)     # DVE sum-pass sink
            sqsc = pers.tile([128, PW1], BF16)     # Act square-pass sink

            def pair_stats(layer, p, blk_ap, ts_out, sq_out, sq_dve=False):
                """blk_ap: view of a pair's pre-BN activations (bf16 SBUF).
                ts_out/sq_out: scratch views with matching free dims. Emits sum
                (DVE tensor_scalar) and sumsq (Act Square or DVE TTR)."""
                nc.vector.tensor_scalar(ts_out, blk_ap, 1.0, 0.0, ALU.mult, ALU.add,
                                        accum_out=sumacc[layer][:, p:p + 1])
                if sq_dve:
                    # x^2 then accumulate, both on DVE (TTR hangs real HW)
                    nc.vector.tensor_tensor(sq_out, blk_ap, blk_ap, ALU.mult)
                    nc.vector.tensor_scalar(sq_out, sq_out, 1.0, 0.0,
                                            ALU.mult, ALU.add,
                                            accum_out=sqacc[layer][:, p:p + 1])
                else:
                    nc.scalar.activation(sq_out, blk_ap, AF.Square,
                                         accum_out=sqacc[layer][:, p:p + 1])

            # ---- helper: region combine + allreduce + scale/shift ----
            def conv_bn(layer, sup_elems, qry_elems, cc_tag):
                st = pers.tile([128, 8], F32, tag=f"stt{layer}")
                sma, sqa = sumacc[layer], sqacc[layer]
                nc.vector.reduce_sum(st[0:64, 0:1], sma[0:64, 0:5], axis=AX.X)
                nc.vector.reduce_sum(st[0:64, 1:2], sma[0:64, 5:18], axis=AX.X)
                nc.vector.reduce_sum(st[64:128, 1:2], sma[64:128, :], axis=AX.X)
                nc.vector.reduce_sum(st[0:64, 2:3], sqa[0:64, 0:5], axis=AX.X)
                nc.vector.reduce_sum(st[0:64, 3:4], sqa[0:64, 5:18], axis=AX.X)
                nc.vector.reduce_sum(st[64:128, 3:4], sqa[64:128, :], axis=AX.X)
                # pack [64, 4]: sup_sum, sup_sq, qry_sum, qry_sq
                pk = pers.tile([64, 8], F32, tag=f"pk{layer}")
                nc.vector.tensor_copy(pk[:, 0:1], st[0:64, 0:1])
                nc.vector.tensor_copy(pk[:, 1:2], st[0:64, 2:3])
                nc.sync.dma_start(pk[:, 4:5], st[64:128, 1:2])
                nc.sync.dma_start(pk[:, 5:6], st[64:128, 3:4])
                nc.vector.tensor_tensor(pk[:, 2:3], st[0:64, 1:2], pk[:, 4:5], ALU.add)
                nc.vector.tensor_tensor(pk[:, 3:4], st[0:64, 3:4], pk[:, 5:6], ALU.add)
                # allgather [64,4] -> [64*N,4], then local sum
                bin_ = dram.tile([64, 4], F32, tag=f"ccin{cc_tag}")
                bout = dram.tile([64 * n_cores, 4], F32, tag=f"ccout{cc_tag}")
                nc.sync.dma_start(bin_[:], pk[:, 0:4])
                nc.gpsimd.collective_compute("AllGather", ALU.bypass, replica_groups=RG,
                                             ins=[bin_.opt()], outs=[bout.opt()])
                gat = pers.tile([64, 4 * n_cores], F32, tag=f"gat{layer}")
                nc.sync.dma_start(gat[:], bout.rearrange("(r p) f -> p r f", p=64))
                red = pers.tile([64, 4], F32, tag=f"red{layer}")
                nc.vector.reduce_sum(red[:], gat.rearrange("p (r f) -> p f r", r=n_cores),
                                     axis=AX.X)
                _bn_scalar_ops(nc, pers, red[:, 0:1], red[:, 1:2], sup_elems,
                               bng128[0:64, layer:layer + 1], bnb128[0:64, layer:layer + 1],
                               sc_t[layer][0:64, 0:1], sh_t[layer][0:64, 0:1], epsc[0:64], f"s{layer}")
                _bn_scalar_ops(nc, pers, red[:, 2:3], red[:, 3:4], qry_elems,
                               bng128[0:64, layer:layer + 1], bnb128[0:64, layer:layer + 1],
                               sc_t[layer][0:64, 1:2], sh_t[layer][0:64, 1:2], epsc[0:64], f"q{layer}")
                # bottom halves are always qry
                nc.sync.dma_start(sc_t[layer][64:128, 0:1], sc_t[layer][0:64, 1:2])
                nc.sync.dma_start(sh_t[layer][64:128, 0:1], sh_t[layer][0:64, 1:2])
                nc.sync.dma_start(sc_t[layer][64:128, 1:2], sc_t[layer][0:64, 1:2])
                nc.sync.dma_start(sh_t[layer][64:128, 1:2], sh_t[layer][0:64, 1:2])

            def bn_apply(layer, in_view, out_view, p):
                """relu(in*sc+sh) -> out, on DVE (two tensor_scalar, 4x when
                bf16/SBUF/packed). Views are whole-pair [128, ...]."""
                col = 0 if p < 5 else 1
                nc.vector.tensor_scalar(out_view, in_view,
                                        sc_t[layer][:, col:col + 1],
                                        sh_t[layer][:, col:col + 1],
                                        ALU.mult, ALU.add)
                nc.vector.tensor_scalar_max(out_view, out_view, 0.0)

            def bn_apply_act(layer, in_view, out_view, p):
                """Fused relu(in*sc+sh) with dtype-quantize on Act (one op)."""
                col = 0 if p < 5 else 1
                nc.scalar.activation(out_view, in_view, AF.Relu,
                                     bias=sh_t[layer][:, col:col + 1],
                                     scale=sc_t[layer][:, col:col + 1])

            # ================= PHASE 1: conv1 + pool + stats =================
            with (
                tc.tile_pool(name="ph1", bufs=1) as ph1,
                tc.tile_pool(name="ph1b", bufs=3) as ph1b,
                tc.tile_pool(name="ph1s", bufs=3) as ph1s,
                tc.tile_pool(name="ph1ps", bufs=6, space="PSUM") as psum,
            ):
                pooled1 = ph1.tile([128, NPAIR * PW1 + PAD1], BF16)
                nc.gpsimd.memset(pooled1[:, NPAIR * PW1:], 0.0)
                # fp8 copy of BN-applied pooled1 for the DoubleRow conv2
                p1q = ph1.tile([128, NPAIR * PW1 + PAD1], FP8)
                nc.gpsimd.memset(p1q[:, NPAIR * PW1:], 0.0)

                for p in range(NPAIR):
                    in27 = ph1b.tile([54, W1], BF16, tag="in27")
                    src_ap = bass.AP(tensor=imgsr_d.ap().tensor,
                                     offset=p * 54 * PLANE,
                                     ap=[[PLANE, 54], [1, W1]])
                    nc.sync.dma_start(in27[:, :], src_ap)
                    # 14 chunks of 504 cols = 6 conv rows each
                    for c in range(14):
                        a = c * 504
                        w = 504 if c < 13 else 336
                        inr = 6 if c < 13 else 4
                        orows = inr // 2
                        ps = psum.tile([128, 512], F32, tag="cps")
                        nc.tensor.matmul(ps[:, :w], w1t[:, :], in27[:, a:a + w])
                        dst = pooled1[:, p * PW1 + 3 * c * 41:
                                      p * PW1 + (3 * c + orows) * 41]
                        if (p * 14 + c) % 5 < 3:
                            # path B: Act deinterleaves psum -> [even;odd] bf16
                            # planes in SBUF, DVE runs two packed 2x TT maxes
                            cb = ph1s.tile([128, 504], BF16, tag="cpb")
                            evod = ps[:, :w].rearrange("p (r c2 t) -> p t r c2",
                                                       t=2, c2=42)[:, :, :, 0:41]
                            cbv = cb[:, :2 * inr * 41].rearrange(
                                "p (t r c2) -> p t r c2", t=2, c2=41)
                            nc.scalar.activation(cbv, evod, AF.Copy)
                            s1 = ph1s.tile([128, 6 * 41], BF16, tag="s1")
                            nc.vector.tensor_tensor(s1[:, :inr * 41],
                                                    cb[:, 0:inr * 41],
                                                    cb[:, inr * 41:2 * inr * 41],
                                                    ALU.max)
                            rr = s1[:, :inr * 41].rearrange("p (r2 t c) -> p t r2 c",
                                                            t=2, c=41)
                            nc.vector.tensor_tensor(
                                dst.rearrange("p (r c) -> p r c", c=41),
                                rr[:, 0], rr[:, 1], ALU.max)
                        else:
                            # path A: single DVE reduce straight from PSUM
                            v5 = ps[:, :w].rearrange("p (r c) -> p r c", r=inr) \
                                [:, :, 0:82].rearrange("p (R rp) (C cp) -> p R C rp cp",
                                                       rp=2, cp=2)
                            nc.vector.tensor_reduce(
                                dst.rearrange("p (r c) -> p r c", c=41),
                                v5, axis=AX.XY, op=ALU.max)
                    # L1 stats for this pair (pre-BN pooled values)
                    pair_stats(0, p, pooled1[:, p * PW1:(p + 1) * PW1],
                               tssc[:, :PW1], sqsc[:, :PW1])

                conv_bn(0, NCORES * 5 * PW1, NCORES * 30 * PW1, 0)

                nc.sync.dma_start(wdr[:], wdr_d[:])
                nc.sync.dma_start(wdr34[:], wdr34_d[:])
                # ---- conv2 (fp8 DoubleRow) + pool (41->39 valid ->19) ----
                c2widths = [492, 492, 492, 82]
                p1q_W = NPAIR * PW1 + PAD1
                for p in range(NPAIR):
                    # fused BN+relu+fp8-quantize for L1 (one Act op per pair)
                    blk = pooled1[:, p * PW1:(p + 1) * PW1]
                    bn_apply_act(0, blk, p1q[:, p * PW1:(p + 1) * PW1], p)
                    if p == 17:
                        nc.gpsimd.memset(p1q[64:128, 17 * PW1:18 * PW1], 0.0)
                    base = p * PW1
                    pstiles = [psum.tile([128, 512], F32, tag="cps", name=f"c2ps{_i}")
                               for _i in range(4)]
                    for gi, (ja, jb) in enumerate(C2_GROUPS):
                        sha = (ja // 3) * 41 + (ja % 3)
                        d = (jb // 3) * 41 + (jb % 3) - sha
                        for c in range(4):
                            a = c * 492
                            w = c2widths[c]
                            if gi < 4:
                                sl = p1q[:, base + a + sha: base + a + sha + w]
                                rhs = bass.AP(tensor=sl.tensor, offset=sl.offset,
                                              ap=[[p1q_W, 128], [d, 2], [1, w]])
                                nc.tensor.matmul(
                                    pstiles[c][:, :w], wdr[:, gi, :, :], rhs,
                                    start=(gi == 0), stop=False,
                                    perf_mode=mybir.MatmulPerfMode.DoubleRow)
                            else:
                                nc.tensor.matmul(
                                    pstiles[c][:, :w], wdr[:, gi, 0, :],
                                    p1q[:, base + a + sha: base + a + sha + w],
                                    start=False, stop=True)
                    for c in range(4):
                        orows = 6 if c < 3 else 1
                        inrows = 12 if c < 3 else 2
                        dst = pooled2[:, p * PW2 + 6 * c * 19:
                                      p * PW2 + (6 * c + orows) * 19]
                        if False:
                            # path B: Act deinterleave -> DVE 2x TT maxes
                            ein = pstiles[c][:, :inrows * 41].rearrange(
                                "p (r c) -> p r c", c=41)[:, :, 0:38].rearrange(
                                "p r (c2 t) -> p t r c2", t=2)
                            cb2 = ph1s.tile([128, 2 * 12 * 19], BF16, tag="cb2")
                            cbv = cb2[:, :2 * inrows * 19].rearrange(
                                "p (t r c2) -> p t r c2", t=2, c2=19)
                            nc.scalar.activation(cbv, ein, AF.Copy)
                            s2 = ph1s.tile([128, 12 * 19], BF16, tag="s2")
                            nc.vector.tensor_tensor(
                                s2[:, :inrows * 19], cb2[:, 0:inrows * 19],
                                cb2[:, inrows * 19:2 * inrows * 19], ALU.max)
                            rr = s2[:, :inrows * 19].rearrange(
                                "p (r2 t c) -> p t r2 c", t=2, c=19)
                            nc.vector.tensor_tensor(
                                dst.rearrange("p (r c) -> p r c", c=19),
                                rr[:, 0], rr[:, 1], ALU.max)
                        else:
                            v5 = pstiles[c][:, :inrows * 41].rearrange(
                                "p (r c) -> p r c", r=inrows)[:, :2 * orows, 0:38] \
                                .rearrange("p (R rp) (C cp) -> p R C rp cp", rp=2, cp=2)
                            nc.vector.tensor_reduce(
                                dst.rearrange("p (r c) -> p r c", r=orows),
                                v5, axis=AX.XY, op=ALU.max)
                    pair_stats(1, p, pooled2[:, p * PW2:(p + 1) * PW2],
                               tssc[:, :PW2], sqsc[:, :PW2])

            conv_bn(1, NCORES * 5 * PW2, NCORES * 30 * PW2, 1)

            nc.sync.dma_start(gw1s[:], gw1s_d[:])
            nc.sync.dma_start(gw1q[:], gw1q_d[:])
            nc.sync.dma_start(gwt[:], gwt_d[:])
            nc.sync.dma_start(fwt[:], fwt_d[:])
            # ================= PHASE 2: conv3, conv4, avgpool =================
            with (
                tc.tile_pool(name="ph2", bufs=1) as ph2,
                tc.tile_pool(name="ph2ps", bufs=8, space="PSUM") as psum,
            ):
                c3buf = ph2.tile([128, NPAIR * PW2 + PAD2], BF16)
                nc.gpsimd.memset(c3buf[:, NPAIR * PW2:], 0.0)
                c17q = ph2.tile([128, NPAIR * PW3 + 36], FP8)
                nc.gpsimd.memset(c17q[:, NPAIR * PW3:], 0.0)
                c4buf = ph2.tile([128, NPAIR * PW3], BF16)

                def conv_layer_dr(srcq, srcq_W, dstbuf, li34, W, Wo, Wc, rowlen,
                                  pre_fn=None, post_fn=None):
                    # fp8 DoubleRow conv: srcq [128, srcq_W] fp8; 5 tap groups
                    for pb in range(0, NPAIR, 4):
                        pe = min(pb + 4, NPAIR)
                        if pre_fn is not None:
                            for pp in range(pb, pe):
                                pre_fn(pp)
                        pst = {pp: psum.tile([128, 512], F32, tag="cps",
                                             name=f"c34ps{pp}") for pp in range(pb, pe)}
                        for gi, (ja, jb) in enumerate(C2_GROUPS):
                            sha = (ja // 3) * rowlen + (ja % 3)
                            d = (jb // 3) * rowlen + (jb % 3) - sha
                            for pp in range(pb, pe):
                                base = pp * W
                                if gi < 4:
                                    sl = srcq[:, base + sha: base + sha + Wc]
                                    rhs = bass.AP(tensor=sl.tensor, offset=sl.offset,
                                                  ap=[[srcq_W, 128], [d, 2], [1, Wc]])
                                    nc.tensor.matmul(
                                        pst[pp][:, :Wc], wdr34[:, li34, gi, :, :], rhs,
                                        start=(gi == 0), stop=False,
                                        perf_mode=mybir.MatmulPerfMode.DoubleRow)
                                else:
                                    nc.tensor.matmul(
                                        pst[pp][:, :Wc], wdr34[:, li34, gi, 0, :],
                                        srcq[:, base + sha: base + sha + Wc],
                                        start=False, stop=True)
                        for pp in range(pb, pe):
                            nc.scalar.activation(dstbuf[:, pp * Wo:pp * Wo + Wc],
                                                 pst[pp][:, :Wc], AF.Copy)
                            if post_fn is not None:
                                post_fn(pp)

                def l2_apply(pp):
                    blk = pooled2[:, pp * PW2:(pp + 1) * PW2]
                    bn_apply_act(1, blk, p2q[:, pp * PW2:(pp + 1) * PW2], pp)
                    if pp == 17:
                        nc.gpsimd.memset(p2q[64:128, 17 * PW2:18 * PW2], 0.0)

                def l3_view(p):
                    return c3buf[:, p * PW2:(p + 1) * PW2].rearrange(
                        "p (r c) -> p r c", r=19)[:, 0:17, 0:17]

                def l3_stats(pp):
                    pair_stats(2, pp, l3_view(pp),
                               tssc[:, :PW3].rearrange("p (r c) -> p r c", r=17),
                               sqsc[:, :PW3].rearrange("p (r c) -> p r c", r=17),
                               sq_dve=True)

                conv_layer_dr(p2q, NPAIR * PW2 + PAD2, c3buf, 0, PW2, PW2,
                              17 * 19, 19, pre_fn=l2_apply, post_fn=l3_stats)
                conv_bn(2, NCORES * 5 * 289, NCORES * 30 * 289, 2)
                for p in range(NPAIR):
                    cv = c17q[:, p * PW3:(p + 1) * PW3].rearrange(
                        "p (r c) -> p r c", r=17)
                    bn_apply_act(2, l3_view(p), cv, p)
                nc.gpsimd.memset(c17q[64:128, 17 * PW3:18 * PW3], 0.0)

                def l4_view(p):
                    return c4buf[:, p * PW3:(p + 1) * PW3].rearrange(
                        "p (r c) -> p r c", r=17)[:, 0:15, 0:15]

                def l4_stats(pp):
                    pair_stats(3, pp, l4_view(pp),
                               tssc[:, :225].rearrange("p (r c) -> p r c", r=15),
                               sqsc[:, :225].rearrange("p (r c) -> p r c", r=15),
                               sq_dve=True)

                conv_layer_dr(c17q, NPAIR * PW3 + 36, c4buf, 1, PW3, PW3,
                              15 * 17, 17, post_fn=l4_stats)
                conv_bn(3, NCORES * 5 * 225, NCORES * 30 * 225, 3)

                # ---- L4 BN apply + avgpool 5x5 -> [64, 9], per pair ----
                featsB = ph2.tile([128, 162], BF16)
                ptmp = ph2.tile([128, 45], F32, tag="ptmp")
                with nc.allow_low_precision("bf16 avgpool partial sums"):
                    for p in range(NPAIR):
                        v = l4_view(p)
                        bn_apply(3, v, v, p)
                        base = p * PW3
                        for half in (0, 1):
                            if half == 1 and p == 17:
                                continue  # pad image unused
                            hs = slice(half * 64, half * 64 + 64)
                            v1 = c4buf[hs, base:base + PW3].rearrange(
                                "p (r c) -> p r c", r=17)[:, 0:15, 0:15].rearrange(
                                "p r (oc k) -> p r oc k", oc=3)
                            nc.vector.reduce_sum(
                                ptmp[hs, :].rearrange("p (r oc) -> p r oc", r=15),
                                v1, axis=AX.X)
                            v2 = ptmp[hs, :].rearrange("p (R k oc) -> p R oc k",
                                                       R=3, k=5, oc=3)
                            if half == 0:
                                dst = feats[0:64, p * 9:(p + 1) * 9].rearrange(
                                    "p (R oc) -> p R oc", R=3)
                                nc.vector.reduce_sum(dst, v2, axis=AX.X)
                            else:
                                dstB = featsB[hs, p * 9:(p + 1) * 9].rearrange(
                                    "p (R oc) -> p R oc", R=3)
                                nc.vector.reduce_sum(dstB, v2, axis=AX.X)
                nc.sync.dma_start(feats[0:64, 162:315], featsB[64:128, 0:153])
                # NOTE: 1/25 avgpool scale is folded into gw1s/gw1q on the host.

            if debug:
                fdbg = pers.tile([66, 324], F32, name="fdbg")
                nc.vector.tensor_copy(fdbg[:], feats[:])
                nc.sync.dma_start(feats_dbg_d[:], fdbg[:])

            # ================= PHASE 3: pairwise g-MLP + f-MLP + loss =================
            with (
                tc.tile_pool(name="ph3", bufs=4) as ph3,
                tc.tile_pool(name="ph3psum", bufs=2, space="PSUM") as ps3,
                tc.tile_pool(name="ph3psg", bufs=3, space="PSUM") as psg,
            ):
                # A[m] [128, 45], B[m] [128, 270]
                A = [ph3.tile([128, 45], BF16, tag=f"A{m}", name=f"A{m}") for m in range(2)]
                Bq = [ph3.tile([128, 270], BF16, tag=f"B{m}", name=f"B{m}") for m in range(2)]
                for m in range(2):
                    pa = ps3.tile([128, 512], F32, tag="abps")
                    nc.tensor.matmul(pa[:, 0:45], gw1s[:, m * 128:(m + 1) * 128], feats[:, 0:45])
                    nc.scalar.activation(A[m][:], pa[:, 0:45], AF.Identity, bias=gb1[:, m:m + 1])
                    pb = ps3.tile([128, 512], F32, tag="abps")
                    nc.tensor.matmul(pb[:, 0:270], gw1q[:, m * 128:(m + 1) * 128], feats[:, 45:315])
                    nc.scalar.activation(Bq[m][:], pb[:, 0:270], AF.Copy)

                # Process 2 queries per iteration as one [128, 810] tile
                # (psum halves at col 0 and 512). Relu+bias split Act/DVE.
                QCH = 405  # one query row-block: 5 s * 81 xy
                ridx = 0

                def make_x1(qp, j):
                    x1 = [ph3.tile([128, 2 * QCH], BF16, tag=f"x1_{j}_{k}",
                                   name=f"x1_{j}_{k}") for k in range(2)]
                    for k in range(2):
                        a_in = A[k][:, None, :, None].to_broadcast((128, 2, 45, 9))
                        b_in = Bq[k][:, qp * 9:(qp + 2) * 9].rearrange(
                            "p (t y) -> p t y", t=2)[:, :, None, :].to_broadcast(
                            (128, 2, 45, 9))
                        out = x1[k][:].rearrange("p (t sx y) -> p t sx y", t=2, y=9)
                        nc.vector.tensor_tensor(out, a_in, b_in, ALU.add)
                        nc.vector.tensor_scalar_max(x1[k][:], x1[k][:], 0.0)
                    return x1

                # 4 queries (2 qpairs) in flight, x1 formed one quad ahead, to
                # cover the per-qpair matmul->relu dependency chain
                quads = [[qp for qp in (qq, qq + 2) if qp < Q]
                         for qq in range(0, Q, 4)]
                xcur = {qp: make_x1(qp, j) for j, qp in enumerate(quads[0])}
                for qi_, qps in enumerate(quads):
                    h = {qp: xcur[qp] for qp in qps}
                    if qi_ + 1 < len(quads):
                        xcur = {qp: make_x1(qp, j)
                                for j, qp in enumerate(quads[qi_ + 1])}
                    for l in range(3):
                        hn = {qp: [ph3.tile([128, 2 * QCH], BF16,
                                            tag=f"h_{j}_{l}_{m}",
                                            name=f"h_{j}_{l}_{m}")
                                   for m in range(2)]
                              for j, qp in enumerate(qps)}
                        for m in range(2):
                            for qp in qps:
                                ps = psg.tile([128, 1024], F32, tag="gps")
                                for ks in range(2):
                                    for qi in range(2):
                                        nc.tensor.matmul(
                                            ps[:, qi * 512: qi * 512 + QCH],
                                            gwt[:, l, ks, m * 128:(m + 1) * 128],
                                            h[qp][ks][:, qi * QCH:(qi + 1) * QCH],
                                            start=(ks == 0), stop=(ks == 1))
                                psv = ps[:].rearrange("p (t c) -> p t c", t=2)[:, :, 0:QCH]
                                hv = hn[qp][m][:].rearrange("p (t c) -> p t c", t=2)
                                if ridx % 10 == 0:
                                    # relu(x + b) on DVE
                                    nc.vector.tensor_scalar(hv, psv, gbt[:, l, m:m + 1],
                                                            0.0, ALU.add, ALU.max)
                                else:
                                    nc.scalar.activation(hv, psv, AF.Relu,
                                                         bias=gbt[:, l, m:m + 1])
                                ridx += 1
                        h = {qp: hn[qp] for qp in qps}
                    for qp in qps:
                        for m in range(2):
                            nc.vector.reduce_sum(
                                xf[:, m, qp * 5:(qp + 2) * 5],
                                h[qp][m].rearrange("p (b e) -> p b e", e=81), axis=AX.X)

                # ---- fbn stats + allreduce ----
                fst = ph3.tile([128, 4], F32, tag="fst")
                sqf = ph3.tile([128, 150], F32, tag="sqf")
                for m in range(2):
                    nc.vector.reduce_sum(fst[:, 2 * m:2 * m + 1], xf[:, m], axis=AX.X)
                    nc.scalar.activation(sqf[:], xf[:, m], AF.Square,
                                         accum_out=fst[:, 2 * m + 1:2 * m + 2])
                fbin = dram.tile([128, 4], F32, tag="ccfin")
                fbout = dram.tile([128 * n_cores, 4], F32, tag="ccfout")
                nc.sync.dma_start(fbin[:], fst[:])
                nc.gpsimd.collective_compute("AllGather", ALU.bypass, replica_groups=RG,
                                             ins=[fbin.opt()], outs=[fbout.opt()])
                fgat = ph3.tile([128, 4 * n_cores], F32, tag="fgat")
                nc.sync.dma_start(fgat[:], fbout.rearrange("(r p) f -> p r f", p=128))
                fred = ph3.tile([128, 4], F32, tag="fred")
                nc.vector.reduce_sum(fred[:], fgat.rearrange("p (r f) -> p f r", r=n_cores),
                                     axis=AX.X)
                fsc = ph3.tile([128, 2], F32, tag="fsc")
                fsh = ph3.tile([128, 2], F32, tag="fsh")
                # vectorized over both m halves: fred cols (sum0, sq0, sum1, sq1)
                fv = fred[:].rearrange("p (m k) -> p k m", k=2)
                ft = ph3.tile([128, 6], F32, tag="fbns")
                fmean, fex2, fvar = ft[:, 0:2], ft[:, 2:4], ft[:, 4:6]
                nc.vector.tensor_scalar_mul(fmean, fv[:, 0], 1.0 / 1200.0)
                nc.vector.tensor_scalar_mul(fex2, fv[:, 1], 1.0 / 1200.0)
                nc.vector.tensor_tensor(fvar, fmean, fmean, ALU.mult)
                nc.vector.tensor_tensor(fvar, fex2, fvar, ALU.subtract)
                nc.scalar.activation(fvar, fvar, AF.Sqrt, bias=epsc[:])
                nc.vector.reciprocal(fvar, fvar)
                nc.vector.tensor_tensor(fsc[:], fbng[:], fvar, ALU.mult)
                nc.vector.tensor_tensor(fmean, fmean, fsc[:], ALU.mult)
                nc.vector.tensor_tensor(fsh[:], fbnb[:], fmean, ALU.subtract)

                if debug:
                    nc.sync.dma_start(xf_dbg_d[:], xf[:])

                # ---- f-MLP on [*, 150] ----
                y = [ph3.tile([128, 150], BF16, tag=f"y{m}", name=f"y{m}") for m in range(2)]
                for m in range(2):
                    nc.scalar.activation(y[m][:], xf[:, m], AF.Identity,
                                         bias=fsh[:, m:m + 1], scale=fsc[:, m:m + 1])
                for l in range(2):
                    yn = [ph3.tile([128, 150], BF16, tag=f"yn{l}_{m}", name=f"yn{l}_{m}") for m in range(2)]
                    for m in range(2):
                        ps = ps3.tile([128, 150], F32, tag="abps")
                        nc.tensor.matmul(ps[:], fwt[:, l, 0, m * 128:(m + 1) * 128], y[0][:],
                                         start=True, stop=False)
                        nc.tensor.matmul(ps[:], fwt[:, l, 1, m * 128:(m + 1) * 128], y[1][:],
                                         start=False, stop=True)
                        nc.scalar.activation(yn[m][:], ps[:], AF.Relu, bias=fbt[:, l, m:m + 1])
                    y = yn
                z3 = ph3.tile([64, 150], BF16, tag="z3")
                ps = ps3.tile([128, 150], F32, tag="abps")
                nc.tensor.matmul(ps[0:64, :], fw3[:, 0, :], y[0][:], start=True, stop=False)
                nc.tensor.matmul(ps[0:64, :], fw3[:, 1, :], y[1][:], start=False, stop=True)
                nc.scalar.activation(z3[:], ps[0:64, :], AF.Relu, bias=fb3[:, 0:1])
                ps4 = ps3.tile([128, 150], F32, tag="abps")
                nc.tensor.matmul(ps4[0:1, :], fw4[:, 0:1], z3[:])
                score = ph3.tile([1, 150], F32, tag="score")
                nc.scalar.activation(score[:], ps4[0:1, :], AF.Sigmoid, bias=fb4[0:1, 0:1])
                dist = ph3.tile([1, 150], F32, tag="dist")
                nc.vector.tensor_scalar(dist[:], score[:], -1.0, 1.0, ALU.mult, ALU.add)
                if debug:
                    nc.sync.dma_start(dist_dbg_d[:], dist[:])

                # ---- margin loss (exact sorted(label*dist)[1] semantics) ----
                v = ph3.tile([1, 150], F32, tag="lv0")
                nc.vector.tensor_tensor(v[:], dist[:], lbl_sb[:], ALU.mult)
                vq = v.rearrange("p (q s) -> p q s", s=S)
                min1 = ph3.tile([1, 30], F32, tag="min1")
                nc.vector.tensor_reduce(min1[:], vq, axis=AX.X, op=ALU.min)
                eq = ph3.tile([1, 150], F32, tag="eq")
                nc.vector.tensor_tensor(eq.rearrange("p (q s) -> p q s", s=S), vq,
                                        min1[:, :, None].to_broadcast((1, 30, 5)), ALU.is_equal)
                cntg = ph3.tile([1, 30], F32, tag="cntg")  # 1.0 if >=2 mins tie
                nc.vector.reduce_sum(cntg[:], eq.rearrange("p (q s) -> p q s", s=S), axis=AX.X)
                nc.vector.tensor_scalar(cntg[:], cntg[:], 1.5, None, ALU.is_ge)
                vx = ph3.tile([1, 150], F32, tag="vx")
                nc.vector.tensor_scalar(vx[:], eq[:], 1e9, None, ALU.mult)
                nc.vector.tensor_tensor(vx[:], vx[:], v[:], ALU.add)
                excl = ph3.tile([1, 30], F32, tag="excl")
                nc.vector.tensor_reduce(excl[:], vx.rearrange("p (q s) -> p q s", s=S),
                                        axis=AX.X, op=ALU.min)
                # min_neg = cntg ? min1 : excl
                nsel = ph3.tile([1, 30], F32, tag="nsel")
                nc.vector.tensor_scalar(nsel[:], cntg[:], -1.0, 1.0, ALU.mult, ALU.add)
                mn = ph3.tile([1, 30], F32, tag="mn")
                nc.vector.tensor_tensor(mn[:], min1[:], cntg[:], ALU.mult)
                nc.vector.tensor_tensor(nsel[:], excl[:], nsel[:], ALU.mult)
                nc.vector.tensor_tensor(mn[:], mn[:], nsel[:], ALU.add)
                t2 = ph3.tile([1, 150], F32, tag="lt2")
                nc.vector.tensor_tensor(t2[:], dist[:], apmask_sb[:], ALU.mult)
                ap_ = ph3.tile([1, 30], F32, tag="ap")
                nc.vector.reduce_sum(ap_[:], t2.rearrange("p (q s) -> p q s", s=S), axis=AX.X)
                dd = ph3.tile([1, 30], F32, tag="dd")
                nc.vector.tensor_tensor(dd[:], ap_[:], mn[:], ALU.subtract)
                lv = ph3.tile([1, 30], F32, tag="lv")
                nc.scalar.activation(lv[:], dd[:], AF.Relu, bias=margin[0:1, 0:1])
                lp = ph3.tile([1, 1], F32, tag="lp")
                nc.vector.reduce_sum(lp[:], lv[:], axis=AX.X)
                nc.sync.dma_start(loss_d[:], lp[:])

    nc.compile()
    return nc


# ---------------------------------------------------------------------------
# host-side preparation
# ---------------------------------------------------------------------------

def _coord():
    ii = np.arange(3, dtype=np.float32) / 3.0
    c = np.stack([np.broadcast_to(ii[:, None], (3, 3)),
                  np.broadcast_to(ii[None, :], (3, 3))], 0).reshape(2, 9)
    return c


def make_in_maps(inp, n_cores=NCORES):
    p = {k: np.ascontiguousarray(np.asarray(v)) for k, v in inp.items()}
    coord = _coord()
    bf = ml_dtypes.bfloat16
    shared = {}
    w27 = p["w1"].transpose(2, 3, 1, 0).reshape(27, 64).astype(np.float32)
    w1t = np.zeros((54, 128), np.float32)
    w1t[0:27, 0:64] = w27; w1t[27:54, 64:128] = w27
    shared["w1t"] = w1t.astype(bf)
    wct = np.stack([p["w2"], p["w3"], p["w4"]]).transpose(0, 3, 4, 2, 1).reshape(3, 9, 64, 64)
    wct = wct.transpose(2, 0, 1, 3)  # [ci, l, j, co]
    wbd = np.zeros((128, 3, 9, 128), np.float32)
    wbd[0:64, :, :, 0:64] = wct
    wbd[64:128, :, :, 64:128] = wct
    f8 = np.dtype(mybir.dt.np(mybir.dt.float8e4))
    wdr = np.zeros((128, 5, 2, 128), np.float32)
    wdr34 = np.zeros((128, 2, 5, 2, 128), np.float32)
    for g, (ja, jb) in enumerate(C2_GROUPS):
        wdr[:, g, 0, :] = wbd[:, 0, ja, :]
        wdr34[:, 0, g, 0, :] = wbd[:, 1, ja, :]
        wdr34[:, 1, g, 0, :] = wbd[:, 2, ja, :]
        if g < 4:
            wdr[:, g, 1, :] = wbd[:, 0, jb, :]
            wdr34[:, 0, g, 1, :] = wbd[:, 1, jb, :]
            wdr34[:, 1, g, 1, :] = wbd[:, 2, jb, :]
    shared["wdr"] = wdr.astype(f8)
    shared["wdr34"] = wdr34.astype(f8)
    shared["bng"] = np.stack([p[f"bn{i}_g"] for i in range(1, 5)], 1).astype(np.float32)
    shared["bnb"] = np.stack([p[f"bn{i}_b"] for i in range(1, 5)], 1).astype(np.float32)
    # fold the 1/25 avgpool scale into the feature rows (not coord rows)
    gsc = np.ones((66, 1), np.float32); gsc[0:64] = 1.0 / 25.0
    shared["gw1s"] = (p["gw1"][:66] * gsc).astype(bf)
    shared["gw1q"] = (p["gw1"][66:] * gsc).astype(bf)
    shared["gb1t"] = p["gb1"].reshape(2, 128).T.astype(np.float32)
    shared["gwt"] = np.stack([p["gw2"], p["gw3"], p["gw4"]]).reshape(3, 2, 128, 256).transpose(2, 0, 1, 3).astype(bf)
    shared["gbt"] = np.stack([p["gb2"], p["gb3"], p["gb4"]]).reshape(3, 2, 128).transpose(2, 0, 1).astype(np.float32)
    shared["fwt"] = np.stack([p["fw1"], p["fw2"]]).reshape(2, 2, 128, 256).transpose(2, 0, 1, 3).astype(bf)
    shared["fbt"] = np.stack([p["fb1"], p["fb2"]]).reshape(2, 2, 128).transpose(2, 0, 1).astype(np.float32)
    shared["fw3t"] = p["fw3"].reshape(2, 128, 64).transpose(1, 0, 2).astype(bf)
    shared["fb3t"] = p["fb3"].reshape(64, 1).astype(np.float32)
    shared["fw4t"] = p["fw4"].reshape(64, 1).astype(bf)
    shared["fb4t"] = p["fb4"].reshape(1, 1).astype(np.float32)
    shared["fbng"] = p["fbn_g"].reshape(2, 128).T.astype(np.float32)
    shared["fbnb"] = p["fbn_b"].reshape(2, 128).T.astype(np.float32)
    shared["coord45"] = np.tile(coord, (1, 5)).astype(bf)
    shared["coord270"] = np.tile(coord, (1, 30)).astype(bf)

    in_maps = []
    for c in range(n_cores):
        m = dict(shared)
        sup, qry = p["support_x"][c], p["query_x"][c]
        order = [sup[i] for i in range(5)] + [qry[i] for i in range(13)] \
            + [qry[13 + i] for i in range(17)] + [np.zeros((3, 84, 84), np.float32)]
        imgs = np.zeros((36, 3, PLANE), bf)
        imgs[:, :, :7056] = np.stack(order).reshape(36, 3, 7056).astype(bf)
        # host-side im2col replication: row (ky*9 + kx*3 + ch) of image slot i
        # is its channel plane shifted by ky*84+kx  ->  [18 pairs, 54, PLANE]
        imgsr = np.zeros((18, 54, PLANE), bf)
        flat = imgs.reshape(36, 3 * PLANE)
        for ky in range(3):
            for kx in range(3):
                sh = ky * IMGW + kx
                blk = flat[:, np.arange(3)[:, None] * PLANE + sh
                           + np.arange(W1)[None, :]]          # [36, 3, W1]
                r = ky * 9 + kx * 3
                imgsr[:, r:r + 3, :W1] = blk[:18]
                imgsr[:, 27 + r:27 + r + 3, :W1] = blk[18:]
        m["imgsr"] = imgsr
        same = (p["support_y"][c][None, :] == p["query_y"][c][:, None])
        m["lbl"] = (~same).astype(np.float32).reshape(1, 150)
        pos_idx = np.argmax(same, axis=1)
        apm = np.zeros((Q, S), np.float32)
        apm[np.arange(Q), pos_idx] = 1.0
        m["apmask"] = apm.reshape(1, 150)
        in_maps.append(m)
    return in_maps


_NC_CACHE = {}


def kernel(**inputs) -> np.ndarray:
    key = (NCORES, False)
    if key not in _NC_CACHE:
        _NC_CACHE[key] = build_nc(NCORES, debug=False)
    nc = _NC_CACHE[key]
    in_maps = make_in_maps(inputs, NCORES)
    res = run_bass_kernel_spmd(nc, in_maps, core_ids=list(range(NCORES)),
                               trace=bool(int(os.environ.get("KTRACE", "0"))))
    if res.exec_time_ns is not None:
        print(f"HW exec time: {res.exec_time_ns} ns")
    total = np.float64(sum(np.float64(r["loss_part"][0, 0]) for r in res.results))
    return np.asarray(total / NCORES, dtype=np.float32)


if __name__ == "__main__":
    d = np.load("/root/problem/ref_inputs.npz")
    inp = {k: d[k] for k in d.files}
    out = kernel(**inp)
    ref = np.load("/root/problem/ref_out.npy")
    print("kernel:", out, "ref:", ref, "rel err:", abs(out - ref) / max(abs(ref), 1e-12))
